# revision 11
# baseline (speedup 1.0000x reference)
"""TRN2 Bass kernel for nn_Attention (Show-Attend-Tell soft attention).

Reference computation (per example b, X_b = encoder_out[b] [P, E]):
  att1 = X_b @ W_enc + b_enc            # [P, A]
  att2 = d_b @ W_dec + b_dec            # [A]
  att  = relu(att1 + att2) @ W_full     # [P]   (+ b_full, softmax-invariant)
  alpha = softmax(att)                  # [P]
  awe   = alpha @ X_b                   # [E]
Returns (awe [B, E], alpha [B, P]).

Sharding: data parallel over batch, 16 examples per core, weights replicated.

Device strategy per core:
  - X is shipped in two layouts: transposed (xt, [16, E, P]) for the big
    matmul (the e-contraction must sit on SBUF partitions) and natural
    (xn, [16, P, E]) for the weighted sum (p-contraction).
  - Matmuls run as float32r (TF32-like: 1 cycle/row for N>=256, ~1e-4
    max rel err, no input conversion needed) or bf16 via the dtype knobs.
  - att1 is computed transposed ([a, p]) with W_enc stationary so the
    per-partition bias (b_enc + b_dec + att2) add and relu fuse into one
    DVE tensor_scalar op per example, and the W_full contraction is an
    accumulating K=128 matmul.
  - softmax runs rowwise on a [16, P] tile (reduce_max(negate) ->
    exp with accum_out -> reciprocal -> scale).
  - alpha is PE-transposed (tiny) so the weighted sum runs with alpha
    stationary, streaming natural X at N=512.
"""
import sys

for _p in ("/opt/trn_rl_repo", "/opt/pypackages"):
    if _p not in sys.path:
        sys.path.append(_p)

import numpy as np
from contextlib import ExitStack

import concourse.bass as bass
import concourse.tile as tile
import concourse.mybir as mybir
from concourse import bacc
from concourse.bass_utils import run_bass_kernel_spmd

B, P, E, A, D = 128, 196, 2048, 512, 512
NCORES = 8
BB = B // NCORES            # 16 examples per core
NE = E // 128               # 16 e-chunks
NA = A // 128               # 4 a-chunks
ND = D // 128               # 4 d-chunks
GRP = 2                     # examples per step-1 matmul (N = 392 <= 512)
NGRP = BB // GRP
PCH = [(0, 128), (128, 68)]  # p-dim chunks (start, size)

F32R = mybir.dt.float32r
F32 = mybir.dt.float32
BF16 = mybir.dt.bfloat16

# Dtype knobs: xt/w_enc feed the step-1 matmul; xn feeds the weighted sum.
XT_DT = F32R
XN_DT = F32R

_CACHE = {}


def _np_dt(dt):
    if dt == BF16:
        import ml_dtypes
        return ml_dtypes.bfloat16
    return np.float32


def _mm(ap, want):
    """View an fp32 AP as f32r for matmul when the knob asks for f32r."""
    if want == F32R and ap.dtype == F32:
        return ap.bitcast(F32R)
    return ap


def build_program():
    nc = bacc.Bacc("TRN2", target_bir_lowering=False, debug=False,
                   num_devices=NCORES)

    xt_d = nc.dram_tensor("xt", [E, BB, P], XT_DT, kind="ExternalInput").ap()
    xn_d = nc.dram_tensor("xn", [BB, P, E], XN_DT, kind="ExternalInput").ap()
    wenc_d = nc.dram_tensor("w_enc", [E, A], XT_DT, kind="ExternalInput").ap()
    wdec_d = nc.dram_tensor("w_dec", [D, A], F32R, kind="ExternalInput").ap()
    dht_d = nc.dram_tensor("dh_t", [D, BB], F32R, kind="ExternalInput").ap()
    beb_d = nc.dram_tensor("b_eb", [A, 1], F32, kind="ExternalInput").ap()
    wful_d = nc.dram_tensor("w_full", [A, 1], F32R, kind="ExternalInput").ap()
    id16_d = nc.dram_tensor("id16", [BB, BB], F32, kind="ExternalInput").ap()

    awe_d = nc.dram_tensor("awe", [BB, E], F32, kind="ExternalOutput").ap()
    alpha_d = nc.dram_tensor("alpha", [BB, P], F32, kind="ExternalOutput").ap()
    # DRAM scratch for gathering per-group att rows (SBUF->SBUF DMA with a
    # reshaping AP silently corrupts data, so bounce through DRAM instead)
    att_scr = nc.dram_tensor("att_scr", [1, BB * P], F32).ap()

    with tile.TileContext(nc) as tc, ExitStack() as ctx:
        cpool = ctx.enter_context(tc.tile_pool(name="const", bufs=1))
        spool = ctx.enter_context(tc.tile_pool(name="small", bufs=1))
        xgpool = ctx.enter_context(tc.tile_pool(name="xg", bufs=2))
        rtpool = ctx.enter_context(tc.tile_pool(name="rt", bufs=2))
        xnpool = ctx.enter_context(tc.tile_pool(name="xn", bufs=4))
        awepool = ctx.enter_context(tc.tile_pool(name="awe", bufs=3))

        # ---- constants ----
        wt = []
        for k in range(NE):
            t = cpool.tile([128, A], XT_DT, tag=f"wenc{k}")
            nc.sync.dma_start(t[:], wenc_d[k * 128:(k + 1) * 128, :])
            wt.append(t)
        wdec = []
        for k in range(ND):
            t = cpool.tile([128, A], F32R, tag=f"wdec{k}")
            nc.sync.dma_start(t[:], wdec_d[k * 128:(k + 1) * 128, :])
            wdec.append(t)
        dht = []
        for k in range(ND):
            t = cpool.tile([128, BB], F32R, tag=f"dht{k}")
            nc.sync.dma_start(t[:], dht_d[k * 128:(k + 1) * 128, :])
            dht.append(t)
        beb = []
        wful = []
        for a in range(NA):
            t = cpool.tile([128, 1], F32, tag=f"beb{a}")
            nc.sync.dma_start(t[:], beb_d[a * 128:(a + 1) * 128, :])
            beb.append(t)
            t2 = cpool.tile([128, 1], F32R, tag=f"wful{a}")
            nc.sync.dma_start(t2[:], wful_d[a * 128:(a + 1) * 128, :])
            wful.append(t2)
        id16 = cpool.tile([BB, BB], F32)
        nc.sync.dma_start(id16[:], id16_d[:])

        # ---- stage A: bias_t[a] = att2^T + (b_enc + b_dec), [128, BB] ----
        bias_t = []
        with tc.tile_pool(name="psA", bufs=2, space="PSUM") as ppa:
            for a in range(NA):
                ps = ppa.tile([128, BB], F32)
                for k in range(ND):
                    nc.tensor.matmul(ps[:], wdec[k][:, a * 128:(a + 1) * 128],
                                     dht[k][:], start=(k == 0),
                                     stop=(k == ND - 1))
                bt = spool.tile([128, BB], F32, tag=f"bias{a}")
                nc.vector.tensor_scalar_add(bt[:], ps[:], beb[a][:])
                bias_t.append(bt)

        # ---- stage B: att rows ----
        att_sb = spool.tile([BB, P], F32, tag="att")
        with tc.tile_pool(name="psB", bufs=3, space="PSUM") as ppb, \
             tc.tile_pool(name="psR", bufs=2, space="PSUM") as ppr:
            for g in range(NGRP):
                n = GRP * P
                xg = []
                for k in range(NE):
                    t = xgpool.tile([128, n], XT_DT, tag=f"xg{k}")
                    src = xt_d[k * 128:(k + 1) * 128,
                               g * GRP:(g + 1) * GRP, :]
                    nc.sync.dma_start(t[:], src)
                    xg.append(t)
                rts = []
                for a in range(NA):
                    ps1 = ppb.tile([128, n], F32)
                    for k in range(NE):
                        nc.tensor.matmul(ps1[:], wt[k][:, a * 128:(a + 1) * 128],
                                         xg[k][:], start=(k == 0),
                                         stop=(k == NE - 1))
                    rt = rtpool.tile([128, n], F32R, tag=f"rt{a}")
                    for x in range(GRP):
                        nc.vector.tensor_scalar(
                            rt[:, x * P:(x + 1) * P],
                            ps1[:, x * P:(x + 1) * P],
                            bias_t[a][:, g * GRP + x:g * GRP + x + 1],
                            0.0,
                            op0=mybir.AluOpType.add,
                            op1=mybir.AluOpType.max)
                    rts.append(rt)
                psr = ppr.tile([1, n], F32)
                for a in range(NA):
                    nc.tensor.matmul(psr[:], wful[a][:], rts[a][:],
                                     start=(a == 0), stop=(a == NA - 1))
                # engine APs must start at partition 0/32/64/96, so att rows
                # are scattered through a DRAM bounce from a partition-0 stage
                stg = rtpool.tile([1, n], F32, tag="attstg")
                nc.vector.tensor_copy(stg[:], psr[:])
                nc.sync.dma_start(att_scr[0:1, g * n:(g + 1) * n], stg[:])
        nc.sync.dma_start(
            att_sb[:], att_scr[0:1, :].rearrange("o (b p) -> (o b) p", b=BB))

        # ---- stage C: softmax rows of att_sb ----
        neg_mx = spool.tile([BB, 1], F32, tag="negmx")
        nc.vector.tensor_reduce(out=neg_mx[:], in_=att_sb[:],
                                op=mybir.AluOpType.max,
                                axis=mybir.AxisListType.X, negate=True)
        ex_sb = spool.tile([BB, P], F32, tag="exsb")
        sm = spool.tile([BB, 1], F32, tag="sm")
        nc.scalar.activation(ex_sb[:], att_sb[:],
                             mybir.ActivationFunctionType.Exp,
                             bias=neg_mx[:], scale=1.0, accum_out=sm[:])
        rs = spool.tile([BB, 1], F32, tag="rs")
        nc.vector.reciprocal(rs[:], sm[:])
        alpha_sb = spool.tile([BB, P], F32, tag="alpha")
        nc.vector.tensor_scalar_mul(alpha_sb[:], ex_sb[:], rs[:])
        nc.sync.dma_start(alpha_d[:], alpha_sb[:])

        # ---- stage D: alpha^T via PE transpose ----
        at = []
        with tc.tile_pool(name="psD", bufs=2, space="PSUM") as ppd:
            for (p0, psz) in PCH:
                pst = ppd.tile([psz, BB], F32)
                nc.tensor.transpose(pst[:], alpha_sb[:, p0:p0 + psz], id16[:])
                t = spool.tile([psz, BB], _at_dt(), tag=f"at{p0}")
                nc.vector.tensor_copy(t[:], pst[:])
                at.append(t)

        # ---- stage E: awe rows ----
        with tc.tile_pool(name="psE", bufs=2, space="PSUM") as ppe:
            for x in range(BB):
                xn = []
                for ci, (p0, psz) in enumerate(PCH):
                    t = xnpool.tile([psz, E], XN_DT, tag=f"xn{ci}")
                    nc.sync.dma_start(t[:], xn_d[x, p0:p0 + psz, :])
                    xn.append(t)
                ps5 = ppe.tile([1, E], F32)
                for nt in range(4):
                    for ci in range(2):
                        nc.tensor.matmul(
                            ps5[0:1, nt * 512:(nt + 1) * 512],
                            at[ci][:, x:x + 1],
                            xn[ci][:, nt * 512:(nt + 1) * 512],
                            start=(ci == 0), stop=(ci == 1))
                stg = awepool.tile([1, E], F32, tag="awestg")
                if x % 2 == 0:
                    nc.vector.tensor_copy(stg[:], ps5[:])
                else:
                    nc.scalar.copy(stg[:], ps5[:])
                nc.sync.dma_start(awe_d[x:x + 1, :], stg[:])

    nc.compile()
    return nc


def _at_dt():
    """SBUF dtype for the transposed-alpha tiles: bf16 when the weighted
    sum runs in bf16, else fp32 (bitcast to f32r at the matmul)."""
    return BF16 if XN_DT == BF16 else F32R


def kernel(encoder_out, decoder_hidden, W_enc, b_enc, W_dec, b_dec,
           W_full, b_full):
    encoder_out = np.asarray(encoder_out, dtype=np.float32)
    decoder_hidden = np.asarray(decoder_hidden, dtype=np.float32)
    W_enc = np.asarray(W_enc, dtype=np.float32)
    b_enc = np.asarray(b_enc, dtype=np.float32)
    W_dec = np.asarray(W_dec, dtype=np.float32)
    b_dec = np.asarray(b_dec, dtype=np.float32)
    W_full = np.asarray(W_full, dtype=np.float32)

    if "nc" not in _CACHE:
        _CACHE["nc"] = build_program()
    nc = _CACHE["nc"]

    xt_np = _np_dt(XT_DT)
    xn_np = _np_dt(XN_DT)
    w_enc_h = np.ascontiguousarray(W_enc.astype(xt_np))
    w_dec_h = np.ascontiguousarray(W_dec)
    b_eb = (b_enc + b_dec).astype(np.float32).reshape(A, 1)
    w_full_h = W_full.reshape(A, 1)
    id16 = np.eye(BB, dtype=np.float32)

    in_maps = []
    for c in range(NCORES):
        sl = slice(c * BB, (c + 1) * BB)
        xc = encoder_out[sl]                       # [BB, P, E] fp32
        in_maps.append({
            "xt": np.ascontiguousarray(xc.transpose(2, 0, 1).astype(xt_np)),
            "xn": np.ascontiguousarray(xc.astype(xn_np)),
            "w_enc": w_enc_h,
            "w_dec": w_dec_h,
            "dh_t": np.ascontiguousarray(decoder_hidden[sl].T),
            "b_eb": b_eb,
            "w_full": w_full_h,
            "id16": id16,
        })

    res = run_bass_kernel_spmd(nc, in_maps, core_ids=list(range(NCORES)))
    awe = np.concatenate([r["awe"] for r in res.results], axis=0)
    alpha = np.concatenate([r["alpha"] for r in res.results], axis=0)
    return awe, alpha


# revision 17
# speedup vs baseline: 1.4859x; 1.4859x over previous
"""TRN2 Bass kernel for nn_Attention (Show-Attend-Tell soft attention).

Reference computation (per example b, X_b = encoder_out[b] [P, E]):
  att1 = X_b @ W_enc + b_enc            # [P, A]
  att2 = d_b @ W_dec + b_dec            # [A]
  att  = relu(att1 + att2) @ W_full     # [P]   (+ b_full, softmax-invariant)
  alpha = softmax(att)                  # [P]
  awe   = alpha @ X_b                   # [E]
Returns (awe [B, E], alpha [B, P]).

Sharding: data parallel over batch, 16 examples per core, weights replicated.

Device strategy per core:
  - X is shipped in two layouts: transposed (xt, [16, E, P]) for the big
    matmul (the e-contraction must sit on SBUF partitions) and natural
    (xn, [16, P, E]) for the weighted sum (p-contraction).
  - Matmuls run as float32r (TF32-like: 1 cycle/row for N>=256, ~1e-4
    max rel err, no input conversion needed) or bf16 via the dtype knobs.
  - att1 is computed transposed ([a, p]) with W_enc stationary so the
    per-partition bias (b_enc + b_dec + att2) add and relu fuse into one
    DVE tensor_scalar op per example, and the W_full contraction is an
    accumulating K=128 matmul.
  - softmax runs rowwise on a [16, P] tile (reduce_max(negate) ->
    exp with accum_out -> reciprocal -> scale).
  - alpha is PE-transposed (tiny) so the weighted sum runs with alpha
    stationary, streaming natural X at N=512.
"""
import sys

for _p in ("/opt/trn_rl_repo", "/opt/pypackages"):
    if _p not in sys.path:
        sys.path.append(_p)

import numpy as np
from contextlib import ExitStack

import concourse.bass as bass
import concourse.tile as tile
import concourse.mybir as mybir
from concourse import bacc
from concourse.bass_utils import run_bass_kernel_spmd

B, P, E, A, D = 128, 196, 2048, 512, 512
NCORES = 8
BB = B // NCORES            # 16 examples per core
NE = E // 128               # 16 e-chunks
NA = A // 128               # 4 a-chunks
ND = D // 128               # 4 d-chunks
GRP = 2                     # examples per step-1 matmul (N = 392 <= 512)
NGRP = BB // GRP
PCH = [(0, 128), (128, 68)]  # p-dim chunks (start, size)

F32R = mybir.dt.float32r
F32 = mybir.dt.float32
BF16 = mybir.dt.bfloat16

# Dtype knobs: xt/w_enc feed the step-1 matmul; xn feeds the weighted sum.
# bf16 runs 1 cycle/row on the PE (f32r measured ~2 cycles/row on HW) and
# halves the HBM traffic; error lands ~1e-3 relative instead of ~1.5e-4.
XT_DT = BF16
XN_DT = BF16

_CACHE = {}


def _np_dt(dt):
    if dt == BF16:
        import ml_dtypes
        return ml_dtypes.bfloat16
    return np.float32


def _mm(ap, want):
    """View an fp32 AP as f32r for matmul when the knob asks for f32r."""
    if want == F32R and ap.dtype == F32:
        return ap.bitcast(F32R)
    return ap


def build_program():
    nc = bacc.Bacc("TRN2", target_bir_lowering=False, debug=False,
                   num_devices=NCORES)

    xt_d = nc.dram_tensor("xt", [E, BB, P], XT_DT, kind="ExternalInput").ap()
    xn_d = nc.dram_tensor("xn", [BB, P, E], XN_DT, kind="ExternalInput").ap()
    wenc_d = nc.dram_tensor("w_enc", [E, A], XT_DT, kind="ExternalInput").ap()
    wdec_d = nc.dram_tensor("w_dec", [D, A], F32R, kind="ExternalInput").ap()
    dht_d = nc.dram_tensor("dh_t", [D, BB], F32R, kind="ExternalInput").ap()
    beb_d = nc.dram_tensor("b_eb", [A, 1], F32, kind="ExternalInput").ap()
    wful_d = nc.dram_tensor("w_full", [A, 1], F32R, kind="ExternalInput").ap()
    id16_d = nc.dram_tensor("id16", [BB, BB], F32, kind="ExternalInput").ap()

    awe_d = nc.dram_tensor("awe", [BB, E], F32, kind="ExternalOutput").ap()
    alpha_d = nc.dram_tensor("alpha", [BB, P], F32, kind="ExternalOutput").ap()
    # DRAM scratch for gathering per-group att rows (SBUF->SBUF DMA with a
    # reshaping AP silently corrupts data, so bounce through DRAM instead)
    att_scr = nc.dram_tensor("att_scr", [1, BB * P], F32).ap()

    with tile.TileContext(nc) as tc, ExitStack() as ctx:
        cpool = ctx.enter_context(tc.tile_pool(name="const", bufs=1))
        spool = ctx.enter_context(tc.tile_pool(name="small", bufs=1))
        xgpool = ctx.enter_context(tc.tile_pool(name="xg", bufs=2))
        rtpool = ctx.enter_context(tc.tile_pool(name="rt", bufs=2))
        xnpool = ctx.enter_context(tc.tile_pool(name="xn", bufs=2))
        awepool = ctx.enter_context(tc.tile_pool(name="awe", bufs=3))

        # ---- constants ----
        wt = []
        for k in range(NE):
            t = cpool.tile([128, A], XT_DT, tag=f"wenc{k}")
            nc.sync.dma_start(t[:], wenc_d[k * 128:(k + 1) * 128, :])
            wt.append(t)
        wdec = []
        for k in range(ND):
            t = cpool.tile([128, A], F32R, tag=f"wdec{k}")
            nc.sync.dma_start(t[:], wdec_d[k * 128:(k + 1) * 128, :])
            wdec.append(t)
        dht = []
        for k in range(ND):
            t = cpool.tile([128, BB], F32R, tag=f"dht{k}")
            nc.sync.dma_start(t[:], dht_d[k * 128:(k + 1) * 128, :])
            dht.append(t)
        beb = []
        wful = []
        for a in range(NA):
            t = cpool.tile([128, 1], F32, tag=f"beb{a}")
            nc.sync.dma_start(t[:], beb_d[a * 128:(a + 1) * 128, :])
            beb.append(t)
            t2 = cpool.tile([128, 1], F32R, tag=f"wful{a}")
            nc.sync.dma_start(t2[:], wful_d[a * 128:(a + 1) * 128, :])
            wful.append(t2)
        id16 = cpool.tile([BB, BB], F32)
        nc.sync.dma_start(id16[:], id16_d[:])

        # ---- stage A: bias_t[a] = att2^T + (b_enc + b_dec), [128, BB] ----
        bias_t = []
        with tc.tile_pool(name="psA", bufs=2, space="PSUM") as ppa:
            for a in range(NA):
                ps = ppa.tile([128, BB], F32)
                for k in range(ND):
                    nc.tensor.matmul(ps[:], wdec[k][:, a * 128:(a + 1) * 128],
                                     dht[k][:], start=(k == 0),
                                     stop=(k == ND - 1))
                bt = spool.tile([128, BB], F32, tag=f"bias{a}")
                nc.vector.tensor_scalar_add(bt[:], ps[:], beb[a][:])
                bias_t.append(bt)

        # ---- stage B: att rows ----
        att_sb = spool.tile([BB, P], F32, tag="att")
        with tc.tile_pool(name="psB", bufs=3, space="PSUM") as ppb, \
             tc.tile_pool(name="psR", bufs=2, space="PSUM") as ppr:
            for g in range(NGRP):
                n = GRP * P
                xg = []
                for k in range(NE):
                    t = xgpool.tile([128, n], XT_DT, tag=f"xg{k}")
                    src = xt_d[k * 128:(k + 1) * 128,
                               g * GRP:(g + 1) * GRP, :]
                    nc.sync.dma_start(t[:], src)
                    xg.append(t)
                rts = []
                for a in range(NA):
                    ps1 = ppb.tile([128, n], F32)
                    for k in range(NE):
                        nc.tensor.matmul(ps1[:], wt[k][:, a * 128:(a + 1) * 128],
                                         xg[k][:], start=(k == 0),
                                         stop=(k == NE - 1))
                    rt = rtpool.tile([128, n], F32R, tag=f"rt{a}")
                    for x in range(GRP):
                        nc.vector.tensor_scalar(
                            rt[:, x * P:(x + 1) * P],
                            ps1[:, x * P:(x + 1) * P],
                            bias_t[a][:, g * GRP + x:g * GRP + x + 1],
                            0.0,
                            op0=mybir.AluOpType.add,
                            op1=mybir.AluOpType.max)
                    rts.append(rt)
                psr = ppr.tile([1, n], F32)
                for a in range(NA):
                    nc.tensor.matmul(psr[:], wful[a][:], rts[a][:],
                                     start=(a == 0), stop=(a == NA - 1))
                # engine APs must start at partition 0/32/64/96, so att rows
                # are scattered through a DRAM bounce from a partition-0 stage
                stg = rtpool.tile([1, n], F32, tag="attstg")
                nc.vector.tensor_copy(stg[:], psr[:])
                nc.sync.dma_start(att_scr[0:1, g * n:(g + 1) * n], stg[:])
        nc.sync.dma_start(
            att_sb[:], att_scr[0:1, :].rearrange("o (b p) -> (o b) p", b=BB))

        # ---- stage C: softmax rows of att_sb ----
        neg_mx = spool.tile([BB, 1], F32, tag="negmx")
        nc.vector.tensor_reduce(out=neg_mx[:], in_=att_sb[:],
                                op=mybir.AluOpType.max,
                                axis=mybir.AxisListType.X, negate=True)
        ex_sb = spool.tile([BB, P], F32, tag="exsb")
        sm = spool.tile([BB, 1], F32, tag="sm")
        nc.scalar.activation(ex_sb[:], att_sb[:],
                             mybir.ActivationFunctionType.Exp,
                             bias=neg_mx[:], scale=1.0, accum_out=sm[:])
        rs = spool.tile([BB, 1], F32, tag="rs")
        nc.vector.reciprocal(rs[:], sm[:])
        alpha_sb = spool.tile([BB, P], F32, tag="alpha")
        nc.vector.tensor_scalar_mul(alpha_sb[:], ex_sb[:], rs[:])
        nc.sync.dma_start(alpha_d[:], alpha_sb[:])

        # ---- stage D: alpha^T via PE transpose ----
        at = []
        with tc.tile_pool(name="psD", bufs=2, space="PSUM") as ppd:
            for (p0, psz) in PCH:
                pst = ppd.tile([psz, BB], F32)
                nc.tensor.transpose(pst[:], alpha_sb[:, p0:p0 + psz], id16[:])
                t = spool.tile([psz, BB], _at_dt(), tag=f"at{p0}")
                nc.vector.tensor_copy(t[:], pst[:])
                at.append(t)

        # ---- stage E: awe rows, 4 examples concurrent via PE column tiling ----
        with tc.tile_pool(name="psE", bufs=2, space="PSUM") as ppe:
            for g4 in range(BB // 4):
                xns = []
                for j in range(4):
                    x = g4 * 4 + j
                    xn = []
                    for ci, (p0, psz) in enumerate(PCH):
                        t = xnpool.tile([psz, E], XN_DT, tag=f"xn{ci}_{j}")
                        nc.sync.dma_start(t[:], xn_d[x, p0:p0 + psz, :])
                        xn.append(t)
                    xns.append(xn)
                ps5 = ppe.tile([128, E], F32)
                for nt in range(4):
                    for j in range(4):
                        x = g4 * 4 + j
                        for ci in range(2):
                            nc.tensor.matmul(
                                ps5[32 * j:32 * j + 1,
                                    nt * 512:(nt + 1) * 512],
                                at[ci][:, x:x + 1],
                                xns[j][ci][:, nt * 512:(nt + 1) * 512],
                                start=(ci == 0), stop=(ci == 1),
                                tile_position=(0, 32 * j))
                # evacuate rows {0,32,64,96} lane-aligned (engines cannot
                # cross or stride partitions); the DMA gathers them to DRAM
                stg = awepool.tile([128, E], F32, tag="awestg")
                for j in range(4):
                    cp = nc.vector.tensor_copy if j % 2 == 0 else nc.scalar.copy
                    cp(stg[32 * j:32 * j + 1, :], ps5[32 * j:32 * j + 1, :])
                nc.sync.dma_start(
                    awe_d[g4 * 4:(g4 + 1) * 4, :],
                    stg[:].rearrange("(j o) e -> j o e", o=32)[:, 0:1, :])

    nc.compile()
    return nc


def _at_dt():
    """SBUF dtype for the transposed-alpha tiles: bf16 when the weighted
    sum runs in bf16, else fp32 (bitcast to f32r at the matmul)."""
    return BF16 if XN_DT == BF16 else F32R


def kernel(encoder_out, decoder_hidden, W_enc, b_enc, W_dec, b_dec,
           W_full, b_full):
    encoder_out = np.asarray(encoder_out, dtype=np.float32)
    decoder_hidden = np.asarray(decoder_hidden, dtype=np.float32)
    W_enc = np.asarray(W_enc, dtype=np.float32)
    b_enc = np.asarray(b_enc, dtype=np.float32)
    W_dec = np.asarray(W_dec, dtype=np.float32)
    b_dec = np.asarray(b_dec, dtype=np.float32)
    W_full = np.asarray(W_full, dtype=np.float32)

    if "nc" not in _CACHE:
        _CACHE["nc"] = build_program()
    nc = _CACHE["nc"]

    xt_np = _np_dt(XT_DT)
    xn_np = _np_dt(XN_DT)
    w_enc_h = np.ascontiguousarray(W_enc.astype(xt_np))
    w_dec_h = np.ascontiguousarray(W_dec)
    b_eb = (b_enc + b_dec).astype(np.float32).reshape(A, 1)
    w_full_h = W_full.reshape(A, 1)
    id16 = np.eye(BB, dtype=np.float32)

    in_maps = []
    for c in range(NCORES):
        sl = slice(c * BB, (c + 1) * BB)
        xc = encoder_out[sl]                       # [BB, P, E] fp32
        in_maps.append({
            "xt": np.ascontiguousarray(xc.transpose(2, 0, 1).astype(xt_np)),
            "xn": np.ascontiguousarray(xc.astype(xn_np)),
            "w_enc": w_enc_h,
            "w_dec": w_dec_h,
            "dh_t": np.ascontiguousarray(decoder_hidden[sl].T),
            "b_eb": b_eb,
            "w_full": w_full_h,
            "id16": id16,
        })

    res = run_bass_kernel_spmd(nc, in_maps, core_ids=list(range(NCORES)))
    awe = np.concatenate([r["awe"] for r in res.results], axis=0)
    alpha = np.concatenate([r["alpha"] for r in res.results], axis=0)
    return awe, alpha
